# revision 1
# baseline (speedup 1.0000x reference)
"""Trainium2 Bass kernel for a teacher-forced decoder LSTM + mean CE loss.

Reference computation (per batch row b, steps t=0..T-2):
    x_t   = emb[inpt[b, t]]
    gates = x_t @ W_ih.T + b_ih + h @ W_hh.T + b_hh        # [4H] blocks i,f,g,o
    c'    = sigmoid(f)*c + sigmoid(i)*tanh(g)
    h'    = sigmoid(o)*tanh(c')
    ce_t  = logsumexp(h' @ W_lin.T + b_lin) - (h' @ W_lin.T + b_lin)[y_t]
    loss  = sum_t sum_b ce_t * mask[b, t] / sum(mask)

Strategy (8 cores, data parallel over batch):
  * Embedding folded into a [30, 4H] table T1 = W_ih @ emb.T + biases; the
    per-step input contribution becomes a one-hot (K=30) matmul.
  * State kept transposed and doubled: H = 2h [128, B], Q = 2c [128, B], so
    every sigmoid becomes tanh(x/2) (one ACT table set: exp_and_others).
    The 0.5 factors are folded into W_hh / W_lin host-side; the g-gate rows
    are pre-doubled so one tanh(0.5*x) pass covers all four gates.
  * Cell update via native scalar_tensor_tensor ops:
        A  = (1 + tanh(i/2)) * tanh(g)     = 2*sigmoid(i)*tanh(g)
        B1 = (1 + tanh(f/2)) * Q           = 4*sigmoid(f)*c
        Qn = 0.5*B1 + A                    = 2*c'
        Hn = (1 + tanh(o/2)) * tanh(Qn/2)  = 2*h'
  * Logits (with b_lin added via a K=1 rank-1 matmul) are copied to a big
    SBUF buffer; exp/row-sum/label-dot/log are deferred to one vectorized
    end phase (avoids tiny per-step ops and ACT table switches).
  * Each core returns [128, 2] partial sums; host reduces to the scalar.
"""

import numpy as np

import os as _os

B, T, V, E, H = 4096, 128, 30, 256, 128
NCORES = 8
BC = B // NCORES            # 512 batch rows per core
TS = T - 1                  # 127 recurrent steps
CHUNK = int(_os.environ.get("LSTM_CHUNK", "256"))  # batch chunk per step
A_ENG = _os.environ.get("LSTM_A_ENG", "gpsimd2")   # gpsimd2|gpsimd1|vector
WBUFS = int(_os.environ.get("LSTM_WBUFS", "5"))
SPLIT_ACT = _os.environ.get("LSTM_SPLIT_ACT", "1") == "1"
MERGETC = _os.environ.get("LSTM_MERGETC", "0") == "1"
# storage order of the gate blocks in psum/tnh; host reorders weight blocks.
# reference gate order is (i, f, g, o) = 0..3
GORDER = tuple(int(x) for x in _os.environ.get("LSTM_GORDER", "0,1,2,3").split(","))
IX_I, IX_F, IX_G, IX_O = (GORDER.index(k) for k in range(4))
NCHUNK = BC // CHUNK
TILES_PER_CHUNK = CHUNK // 128   # 2
NTILE = BC // 128           # 4 CE tiles
SCOLS = TS * NTILE          # 508 columns in the S/L bookkeeping buffers
LCOLS = TS * NTILE * V      # 15240 logits columns stored per partition
# column offsets inside the packed consts array [128, CCOLS]
C_T1T = 0                   # [V, 4H]
C_WHHT = C_T1T + 4 * H      # [H, 4H]
C_WLIN = C_WHHT + 4 * H     # [H, V]
C_BLIN = C_WLIN + V         # [1, 2V]
C_ONES = C_BLIN + TILES_PER_CHUNK * V  # [1, H]
C_H0 = C_ONES + H           # [H, BC]
C_Q0 = C_H0 + BC            # [H, BC]
C_MBUF = C_Q0 + BC          # [128, SCOLS]
CCOLS = C_MBUF + SCOLS + 2  # pad
EGROUP = int(_os.environ.get("LSTM_EGROUP", "16"))  # steps per end-phase group
NEG = (TS + EGROUP - 1) // EGROUP  # 8 end-phase groups

_cache = {}


def _build_nc(trace_label=""):
    import concourse.bass as bass
    import concourse.mybir as mybir
    from concourse import bacc
    from concourse.tile import TileContext
    from contextlib import ExitStack

    f32 = mybir.dt.float32
    f32r = mybir.dt.float32r
    AF = mybir.ActivationFunctionType
    ALU = mybir.AluOpType

    nc = bacc.Bacc()

    # ---- DRAM I/O (per core) ----
    # consts packs t1t/whht/wlint/blin2/h0t/q0t/mbuf/ones into one DMA so the
    # kernel prologue has a single wait source (per-instruction wait limits).
    consts_d = nc.dram_tensor("consts", [128, CCOLS], f32r, kind="ExternalInput")
    ohx_d = nc.dram_tensor("ohx", [TS, V, BC], f32r, kind="ExternalInput")
    oym_d = nc.dram_tensor("oym", [TS, NCHUNK, 128, TILES_PER_CHUNK * V], f32, kind="ExternalInput")
    res_d = nc.dram_tensor("res", [128, 2], f32, kind="ExternalOutput")

    with ExitStack() as ctx:
        tc = ctx.enter_context(TileContext(nc))
        singles = ctx.enter_context(tc.tile_pool(name="singles", bufs=1))
        work = ctx.enter_context(tc.tile_pool(name="work", bufs=WBUFS))
        endw = ctx.enter_context(tc.tile_pool(name="endw", bufs=2))
        GBUFS = int(_os.environ.get("LSTM_GBUFS", "2" if CHUNK <= 256 else "1"))
        LBUFS = int(_os.environ.get("LSTM_LBUFS", "2"))
        QPSUM = _os.environ.get("LSTM_QPSUM", "0") == "1"
        gpool = ctx.enter_context(tc.tile_pool(name="gpsum", bufs=GBUFS, space="PSUM"))
        lpool = ctx.enter_context(tc.tile_pool(name="lpsum", bufs=LBUFS, space="PSUM"))
        if QPSUM:
            qpool = ctx.enter_context(tc.tile_pool(name="qpsum", bufs=1, space="PSUM"))

        # ---- persistent SBUF ----
        consts = singles.tile([128, CCOLS], f32r)
        Hs = singles.tile([H, BC], f32r)
        if QPSUM:
            Qs = qpool.tile([H, BC], f32)
        else:
            Qs = singles.tile([H, BC], f32)
        sbufS = singles.tile([128, SCOLS], f32)      # row sums of exp(logits)
        lbig = singles.tile([128, LCOLS], f32)       # stored logits
        res = singles.tile([128, 2], f32)

        nc.sync.dma_start(out=consts, in_=consts_d[:, :])
        t1t = consts[:V, C_T1T:C_T1T + 4 * H]
        whht = consts[:H, C_WHHT:C_WHHT + 4 * H]
        wlint = consts[:H, C_WLIN:C_WLIN + V]
        blin2 = consts[:1, C_BLIN:C_BLIN + TILES_PER_CHUNK * V]
        ones_row = consts[:1, C_ONES:C_ONES + H]
        mbuf = consts[:, C_MBUF:C_MBUF + SCOLS].bitcast(f32)
        nc.vector.tensor_copy(Hs, consts[:H, C_H0:C_H0 + BC])
        nc.vector.tensor_copy(Qs, consts[:H, C_Q0:C_Q0 + BC].bitcast(f32))

        # ---- recurrent loop ----
        ONE_OHX = _os.environ.get("LSTM_ONE_OHX", "0") == "1"
        LOGITS_LAST = _os.environ.get("LSTM_LOGITS_LAST", "0") == "1"
        CE_INLOOP = _os.environ.get("LSTM_CE_INLOOP", "0") == "1"
        ce_groups = []
        done_groups = list(range(NEG))
        lrows_early = singles.tile([128, SCOLS], f32)

        def emit_ce_group(gidx):
            t0 = gidx * EGROUP
            t1 = min(TS, t0 + EGROUP)
            ncols = (t1 - t0) * NTILE * V
            nrows = (t1 - t0) * NTILE
            lsl = lbig[:, t0 * NTILE * V: t0 * NTILE * V + ncols]
            oyg = endw.tile([128, EGROUP * NTILE * V], f32, tag="oyg")
            nc.sync.dma_start(
                out=oyg[:, :ncols],
                in_=oym_d[t0:t1].rearrange("t c p v -> p t c v"))
            es = endw.tile([128, EGROUP * NTILE, V], f32, tag="es")
            essl = es[:, :nrows, :]
            nc.scalar.activation(essl, lsl.rearrange("p (n v) -> p n v", v=V), AF.Exp)
            nc.vector.tensor_reduce(
                out=sbufS[:, t0 * NTILE: t0 * NTILE + nrows], in_=essl,
                axis=mybir.AxisListType.X, op=ALU.add)
            scr = endw.tile([128, EGROUP * NTILE, V], f32, tag="scr")
            nc.gpsimd.tensor_tensor(
                scr[:, :nrows, :].rearrange("p n v -> p (n v)"), lsl,
                oyg[:, :ncols], ALU.mult)
            nc.vector.tensor_reduce(
                out=lrows_early[:, t0 * NTILE: t0 * NTILE + nrows],
                in_=scr[:, :nrows, :], axis=mybir.AxisListType.X, op=ALU.add)

        for t in range(TS):
            tnh_keep = []
            hn_done = []
            if ONE_OHX:
                ohx_full = work.tile([V, BC], f32r, tag="ohxf")
                nc.sync.dma_start(out=ohx_full, in_=ohx_d[t])
            for c in range(NCHUNK):
                cl = slice(c * CHUNK, (c + 1) * CHUNK)
                if ONE_OHX:
                    ohx = ohx_full[:, cl]
                else:
                    ohx = work.tile([V, CHUNK], f32r, tag="ohx")
                    nc.sync.dma_start(out=ohx, in_=ohx_d[t, :, cl])

                # gates: [128, 4, CHUNK] psum; block g = preact (g-gate doubled)
                gp = gpool.tile([128, 4, CHUNK], f32, tag="gp")
                for g in range(4):
                    nc.tensor.matmul(gp[:, g, :], t1t[:, g * H:(g + 1) * H],
                                     ohx, start=True, stop=False)
                    nc.tensor.matmul(gp[:, g, :], whht[:, g * H:(g + 1) * H],
                                     Hs[:, cl], start=False, stop=True)

                tnh = work.tile([128, 4, CHUNK], f32, tag="tnh")
                if SPLIT_ACT:
                    nsplit = 2 if GORDER != (0, 1, 2, 3) else 3
                    nc.scalar.activation(tnh[:, :nsplit, :], gp[:, :nsplit, :], AF.Tanh, scale=0.5)
                    nc.scalar.activation(tnh[:, nsplit:, :], gp[:, nsplit:, :], AF.Tanh, scale=0.5)
                else:
                    nc.scalar.activation(tnh, gp, AF.Tanh, scale=0.5)

                a_t = work.tile([128, CHUNK], f32, tag="a")
                if A_ENG == "vector":
                    nc.vector.scalar_tensor_tensor(
                        out=a_t, in0=tnh[:, IX_I, :], scalar=1.0, in1=tnh[:, IX_G, :],
                        op0=ALU.add, op1=ALU.mult)
                elif A_ENG == "gpsimd1":
                    a1_t = work.tile([128, CHUNK], f32, tag="a1")
                    nc.gpsimd.tensor_tensor(a1_t, tnh[:, IX_I, :], tnh[:, IX_G, :], ALU.mult)
                    nc.vector.tensor_add(a_t, a1_t, tnh[:, IX_G, :])
                elif A_ENG == "gsplit":
                    pass
                else:  # gpsimd2
                    a1_t = work.tile([128, CHUNK], f32, tag="a1")
                    nc.gpsimd.tensor_tensor(a1_t, tnh[:, IX_I, :], tnh[:, IX_G, :], ALU.mult)
                    nc.gpsimd.tensor_tensor(a_t, a1_t, tnh[:, IX_G, :], ALU.add)
                b_t = work.tile([128, CHUNK], f32, tag="b")
                nc.vector.scalar_tensor_tensor(
                    out=b_t, in0=tnh[:, IX_F, :], scalar=1.0, in1=Qs[:, cl],
                    op0=ALU.add, op1=ALU.mult)
                if A_ENG == "gsplit":
                    a1_t = work.tile([128, CHUNK], f32, tag="a1")
                    nc.gpsimd.tensor_tensor(a1_t, tnh[:, IX_I, :], tnh[:, IX_G, :], ALU.mult)
                    q1_t = work.tile([128, CHUNK], f32, tag="q1")
                    nc.vector.scalar_tensor_tensor(
                        out=q1_t, in0=b_t, scalar=0.5, in1=a1_t,
                        op0=ALU.mult, op1=ALU.add)
                    nc.vector.tensor_add(Qs[:, cl], q1_t, tnh[:, IX_G, :])
                else:
                    nc.vector.scalar_tensor_tensor(
                        out=Qs[:, cl], in0=b_t, scalar=0.5, in1=a_t,
                        op0=ALU.mult, op1=ALU.add)
                if not MERGETC:
                    tc_t = work.tile([128, CHUNK], f32, tag="tc")
                    nc.scalar.activation(tc_t, Qs[:, cl], AF.Tanh, scale=0.5)
                    nc.vector.scalar_tensor_tensor(
                        out=Hs[:, cl], in0=tnh[:, IX_O, :], scalar=1.0, in1=tc_t,
                        op0=ALU.add, op1=ALU.mult)
                else:
                    tnh_keep.append(tnh)

                # logits for this chunk: [128, 2, V] (batch partition layout)
                if LOGITS_LAST:
                    hn_done.append(c)
                elif not MERGETC:
                    lp = lpool.tile([128, TILES_PER_CHUNK, V], f32, tag="lp")
                    # per-tile logits first (j2=0 clears the bank's has_written
                    # bits, j2=1 overwrites its own cols), then one rank-1 bias
                    # matmul accumulates b_lin everywhere and closes the group.
                    for j2 in range(TILES_PER_CHUNK):
                        nc.tensor.matmul(
                            lp[:, j2, :],
                            Hs[:, c * CHUNK + j2 * 128: c * CHUNK + (j2 + 1) * 128],
                            wlint, start=(j2 == 0), stop=False,
                            skip_group_check=True)
                    nc.tensor.matmul(lp, ones_row, blin2, start=False, stop=True,
                                     skip_group_check=True)
                if not MERGETC and not LOGITS_LAST:
                    lslice = lbig[:, (t * NTILE + c * TILES_PER_CHUNK) * V:
                                     (t * NTILE + (c + 1) * TILES_PER_CHUNK) * V]
                    nc.vector.tensor_copy(lslice, lp)

            if LOGITS_LAST:
                for c in hn_done:
                    lp = lpool.tile([128, TILES_PER_CHUNK, V], f32, tag="lp")
                    for j2 in range(TILES_PER_CHUNK):
                        nc.tensor.matmul(
                            lp[:, j2, :],
                            Hs[:, c * CHUNK + j2 * 128: c * CHUNK + (j2 + 1) * 128],
                            wlint, start=(j2 == 0), stop=False,
                            skip_group_check=True)
                    nc.tensor.matmul(lp, ones_row, blin2, start=False, stop=True,
                                     skip_group_check=True)
                    lslice = lbig[:, (t * NTILE + c * TILES_PER_CHUNK) * V:
                                     (t * NTILE + (c + 1) * TILES_PER_CHUNK) * V]
                    nc.vector.tensor_copy(lslice, lp)
            if MERGETC:
                tc_t = work.tile([128, BC], f32, tag="tc")
                nc.scalar.activation(tc_t, Qs, AF.Tanh, scale=0.5)
                for c in range(NCHUNK):
                    cl = slice(c * CHUNK, (c + 1) * CHUNK)
                    nc.vector.scalar_tensor_tensor(
                        out=Hs[:, cl], in0=tnh_keep[c][:, 3, :], scalar=1.0,
                        in1=tc_t[:, cl], op0=ALU.add, op1=ALU.mult)
                    lp = lpool.tile([128, TILES_PER_CHUNK, V], f32, tag="lp")
                    for j2 in range(TILES_PER_CHUNK):
                        nc.tensor.matmul(
                            lp[:, j2, :],
                            Hs[:, c * CHUNK + j2 * 128: c * CHUNK + (j2 + 1) * 128],
                            wlint, start=(j2 == 0), stop=False,
                            skip_group_check=True)
                    nc.tensor.matmul(lp, ones_row, blin2, start=False, stop=True,
                                     skip_group_check=True)
                    lslice = lbig[:, (t * NTILE + c * TILES_PER_CHUNK) * V:
                                     (t * NTILE + (c + 1) * TILES_PER_CHUNK) * V]
                    nc.vector.tensor_copy(lslice, lp)

            if CE_INLOOP and (t + 1) % EGROUP == 0:
                gidx = (t + 1) // EGROUP - 1
                emit_ce_group(gidx)
                ce_groups.append(gidx)

        if CE_INLOOP:
            done_groups = [g for g in range(NEG) if g not in ce_groups]

        # ---- end phase: exp / row-sums / label dot / log ----
        lrows = lrows_early
        for gidx in done_groups:
            t0 = gidx * EGROUP
            t1 = min(TS, t0 + EGROUP)
            ncols = (t1 - t0) * NTILE * V
            nrows = (t1 - t0) * NTILE
            lsl = lbig[:, t0 * NTILE * V: t0 * NTILE * V + ncols]
            oyg = endw.tile([128, EGROUP * NTILE * V], f32, tag="oyg")
            nc.sync.dma_start(
                out=oyg[:, :ncols],
                in_=oym_d[t0:t1].rearrange("t c p v -> p t c v"))
            es = endw.tile([128, EGROUP * NTILE, V], f32, tag="es")
            essl = es[:, :nrows, :]
            nc.scalar.activation(essl, lsl.rearrange("p (n v) -> p n v", v=V), AF.Exp)
            nc.vector.tensor_reduce(
                out=sbufS[:, t0 * NTILE: t0 * NTILE + nrows], in_=essl,
                axis=mybir.AxisListType.X, op=ALU.add)
            scr = endw.tile([128, EGROUP * NTILE, V], f32, tag="scr")
            nc.gpsimd.tensor_tensor(
                scr[:, :nrows, :].rearrange("p n v -> p (n v)"), lsl,
                oyg[:, :ncols], ALU.mult)
            nc.vector.tensor_reduce(
                out=lrows[:, t0 * NTILE: t0 * NTILE + nrows], in_=scr[:, :nrows, :],
                axis=mybir.AxisListType.X, op=ALU.add)

        lnb = endw.tile([128, SCOLS], f32, tag="lnb")
        nc.scalar.activation(lnb, sbufS, AF.Ln)
        scr2 = endw.tile([128, SCOLS], f32, tag="scr2")
        nc.vector.tensor_mul(scr2, lnb, mbuf)
        nc.vector.tensor_reduce(out=res[:, 0:1], in_=scr2,
                                axis=mybir.AxisListType.X, op=ALU.add)
        nc.vector.tensor_reduce(out=res[:, 1:2], in_=lrows,
                                axis=mybir.AxisListType.X, op=ALU.add)
        nc.sync.dma_start(out=res_d[:, :], in_=res)

    nc.finalize()
    return nc


def _host_prep(inpt, h0, c0, mask_Y, emb, W_ih, b_ih, W_hh, b_hh, W_lin, b_lin):
    """Build per-core input maps (all fp32, C-contiguous)."""
    f = np.float32
    T1 = W_ih.astype(np.float64) @ emb.astype(np.float64).T \
        + (b_ih + b_hh).astype(np.float64)[:, None]          # [4H, V]
    T1 = T1.astype(f)
    gate_scale = np.ones((4, 1), f)
    gate_scale[2] = 2.0                                       # double g-gate preact
    T1_eff = (T1.reshape(4, H, V) * gate_scale[:, :, None])[list(GORDER)].reshape(4 * H, V)
    w_scale = np.array([0.5, 0.5, 1.0, 0.5], f)[:, None, None]
    Whh_eff = (W_hh.reshape(4, H, H).astype(f) * w_scale)[list(GORDER)].reshape(4 * H, H)
    t1t = np.ascontiguousarray(T1_eff.T)                      # [V, 4H]
    whht = np.ascontiguousarray(Whh_eff.T)                    # [H, 4H]
    wlint = np.ascontiguousarray(0.5 * W_lin.astype(f).T)     # [H, V]
    blin2 = np.ascontiguousarray(
        np.tile(b_lin.astype(f)[None, :], (1, TILES_PER_CHUNK)))  # [1, 2V]

    idx_in = inpt[:, :TS]                                     # [B, TS]
    y = inpt[:, 1:]                                           # [B, TS]
    m = mask_Y[:, :TS].astype(f)                              # [B, TS]

    maps = []
    for k in range(NCORES):
        rows = slice(k * BC, (k + 1) * BC)
        xi = idx_in[rows]                                     # [BC, TS]
        ohx = (xi.T[:, None, :] == np.arange(V, dtype=xi.dtype)[None, :, None])
        ohx = np.ascontiguousarray(ohx.astype(f))             # [TS, V, BC]
        yk = y[rows]                                          # [BC, TS]
        mk = m[rows]                                          # [BC, TS]
        # oym[t, c, p, j2*V + v] = (y[j,t]==v)*m[j,t], j = c*CHUNK + j2*128 + p
        oh_y = (yk[:, :, None] == np.arange(V, dtype=yk.dtype)[None, None, :])
        oh_ym = oh_y.astype(f) * mk[:, :, None]               # [BC, TS, V]
        oym = oh_ym.reshape(NCHUNK, TILES_PER_CHUNK, 128, TS, V)
        oym = np.ascontiguousarray(
            oym.transpose(3, 0, 2, 1, 4).reshape(TS, NCHUNK, 128, TILES_PER_CHUNK * V))
        # mbuf[p, t*NTILE + j] = m[j*128 + p, t]
        mb = mk.reshape(NTILE, 128, TS)
        mbuf = np.ascontiguousarray(mb.transpose(1, 2, 0).reshape(128, SCOLS))
        consts = np.zeros((128, CCOLS), f)
        consts[:V, C_T1T:C_T1T + 4 * H] = t1t
        consts[:H, C_WHHT:C_WHHT + 4 * H] = whht
        consts[:H, C_WLIN:C_WLIN + V] = wlint
        consts[0, C_BLIN:C_BLIN + TILES_PER_CHUNK * V] = blin2[0]
        consts[0, C_ONES:C_ONES + H] = 1.0
        consts[:H, C_H0:C_H0 + BC] = 2.0 * h0[rows].astype(f).T
        consts[:H, C_Q0:C_Q0 + BC] = 2.0 * c0[rows].astype(f).T
        consts[:, C_MBUF:C_MBUF + SCOLS] = mbuf
        maps.append({"consts": consts, "ohx": ohx, "oym": oym})
    return maps


def kernel(inpt, h0, c0, mask_Y, beta, emb, W_ih, b_ih, W_hh, b_hh, W_lin, b_lin,
           _want_results=False, _trace=False):
    from concourse.bass_utils import run_bass_kernel_spmd

    inpt = np.asarray(inpt)
    h0 = np.asarray(h0, np.float32)
    c0 = np.asarray(c0, np.float32)
    mask_Y = np.asarray(mask_Y, np.float32)
    emb = np.asarray(emb, np.float32)
    W_ih = np.asarray(W_ih, np.float32)
    b_ih = np.asarray(b_ih, np.float32)
    W_hh = np.asarray(W_hh, np.float32)
    b_hh = np.asarray(b_hh, np.float32)
    W_lin = np.asarray(W_lin, np.float32)
    b_lin = np.asarray(b_lin, np.float32)

    if "nc" not in _cache:
        _cache["nc"] = _build_nc()
    nc = _cache["nc"]

    in_maps = _host_prep(inpt, h0, c0, mask_Y, emb, W_ih, b_ih, W_hh, b_hh,
                         W_lin, b_lin)
    out = run_bass_kernel_spmd(nc, in_maps, core_ids=list(range(NCORES)),
                               trace=_trace)
    total = 0.0
    for rdict in out.results:
        r = rdict["res"].astype(np.float64)
        total += r[:, 0].sum() - r[:, 1].sum()
    loss = total / np.sum(mask_Y, dtype=np.float64)
    result = np.array(loss, dtype=np.float32)
    if _want_results:
        return result, out
    return result

